# revision 1
# baseline (speedup 1.0000x reference)
"""Gabor-atom additive audio synthesis on 8 Trainium2 NeuronCores.

Math: waveform[t] = sum_n amp_n * exp(-0.5*((t-tau_n)/sigma_n)^2)
                    * cos(2*pi*omega_n*(t-tau_n) + gamma_n*(t-tau_n)^2 + phi_n)
with N=2048 atoms, T=48000 samples (2s @ 24kHz).

Sharding: atoms across 8 cores (256 atoms/core = 2 blocks of 128 partitions).
Per core, time is tiled (F=768). For each (block, tile) the per-element work is:
  - envelope log-arg q and phase-in-cycles y are evaluated as quadratics in
    local-time u via PE matmuls whose stationary rows are bf16 multi-split
    coefficients (guaranteed ~2^-25 relative precision, full PE rate)
  - env' = amp*exp(q) via one ACT Exp (amp folded in as ln(amp))
  - y -> frac = y - round(y) via DVE int32 round + mixed-dtype subtract
  - cos = Sin(2*pi*frac) via one ACT Sin (the +0.25 cycle shift is folded
    into the phase constant so Sin directly yields the cosine)
  - m = env'*cos on GPSIMD; PE reduce (stationary=m chunk, moving=ones)
    accumulates waveform columns into a single PSUM bank [128, 375]
ACT work is batched by table set (Exp... then Sin...) to avoid table thrash.
Host: fp64 coefficient prep, final 8-way partial sum.
"""
import numpy as np
import ml_dtypes
from contextlib import ExitStack

import concourse.bacc as bacc
import concourse.tile as tile
from concourse import mybir
from concourse.bass_utils import run_bass_kernel_spmd
from concourse.tile import add_dep_helper

# ---- problem constants (hardcoded; kernel.py must be self-contained) ----
FS = 24000.0
T = 48000
N_ATOMS = 2048
N_CORES = 8
NYQUIST = FS / 2.0
SIGMA_OFFSET = 1e-3

P = 128                      # partitions / atoms per block
BLOCKS = 2                   # atom blocks per core (256 atoms/core)
F = 768                      # time-tile width (u_max = 767/24000 ~ 0.032 s)
N_TILES = T // F + (1 if T % F else 0)      # 63 (62 full + 384 remainder)
REM = T - (N_TILES - 1) * F                 # 384
CHUNK = 128                  # reduce chunk (output column) width
N_COLS = T // CHUNK          # 375 output columns
KQ = 9                       # envelope matmul contraction rows
KP = 8                       # phase matmul contraction rows
BATCH = 5
ENV_BUFS = 14
FRAC_BUFS = 14
MM_N = 384                   # matmul free-dim chunk (<=512, PSUM bank limit)

f32 = mybir.dt.float32
i32 = mybir.dt.int32
bf16 = mybir.dt.bfloat16
bft = ml_dtypes.bfloat16

_cache = {}


def _bsplit(x, n):
    """Split fp64 array into n bf16 terms summing to ~2^-(9n) rel precision."""
    parts, r = [], np.asarray(x, np.float64).copy()
    for _ in range(n):
        p = r.astype(bft)
        parts.append(p)
        r = r - p.astype(np.float64)
    return parts


def _build_program():
    nc = bacc.Bacc("TRN2", target_bir_lowering=False, debug=False)

    d_movq = nc.dram_tensor("mov_q", [KQ, F], bf16, kind="ExternalInput").ap()
    d_movp = nc.dram_tensor("mov_p", [KP, F], bf16, kind="ExternalInput").ap()
    d_statq = nc.dram_tensor("stat_q", [N_TILES, KQ, BLOCKS * P], bf16,
                             kind="ExternalInput").ap()
    d_statp = nc.dram_tensor("stat_p", [N_TILES, KP, BLOCKS * P], bf16,
                             kind="ExternalInput").ap()
    d_out = nc.dram_tensor("wave", [P, N_COLS], f32, kind="ExternalOutput").ap()

    act_chain = []  # explicit ACT-stream order: batches of Exp, then Sin
    with tile.TileContext(nc) as tc, ExitStack() as ctx:
        consts = ctx.enter_context(tc.tile_pool(name="consts", bufs=1))
        statpool = ctx.enter_context(tc.tile_pool(name="stats", bufs=4))
        envpool = ctx.enter_context(tc.tile_pool(name="env", bufs=ENV_BUFS))
        fracpool = ctx.enter_context(tc.tile_pool(name="frac", bufs=FRAC_BUFS))
        kpool = ctx.enter_context(tc.tile_pool(name="kint", bufs=5))
        sinpool = ctx.enter_context(tc.tile_pool(name="sin", bufs=4))
        mpool = ctx.enter_context(tc.tile_pool(name="m", bufs=4))
        opool = ctx.enter_context(tc.tile_pool(name="ocopy", bufs=1))
        qppool = ctx.enter_context(tc.tile_pool(name="qp", bufs=2, space="PSUM"))
        outpool = ctx.enter_context(tc.tile_pool(name="outp", bufs=1, space="PSUM"))

        t_movq = consts.tile([KQ, F], bf16)
        nc.sync.dma_start(t_movq[:], d_movq[:])
        t_movp = consts.tile([KP, F], bf16)
        nc.gpsimd.dma_start(t_movp[:], d_movp[:])
        t_ones = consts.tile([P, 1], bf16)
        nc.vector.memset(t_ones[:], 1.0)

        p_out = outpool.tile([P, 512], f32)

        def tf(i):  # tile i free width
            return REM if i == N_TILES - 1 else F

        # ---- per-tile stages ----
        def stage_a(i):
            """matmuls -> env (ACT Exp) + frac (DVE). Returns (env, frac) tiles."""
            w = tf(i)
            t_sq = statpool.tile([KQ, BLOCKS * P], bf16, tag="sq")
            nc.sync.dma_start(t_sq[:], d_statq[i])
            t_sp = statpool.tile([KP, BLOCKS * P], bf16, tag="sp")
            nc.sync.dma_start(t_sp[:], d_statp[i])

            p_q = qppool.tile([P, BLOCKS * F], f32, tag="ps")
            p_p = qppool.tile([P, BLOCKS * F], f32, tag="ps")
            # matmul PSUM writes must not straddle a 512-col bank boundary
            for b in range(BLOCKS):
                o = 0
                while o < w:
                    col = b * w + o
                    n = min(w - o, 512 - (col % 512))
                    nc.tensor.matmul(
                        p_q[:, col: col + n],
                        t_sq[:, b * P:(b + 1) * P], t_movq[:, o:o + n],
                        start=True, stop=True)
                    nc.tensor.matmul(
                        p_p[:, col: col + n],
                        t_sp[:, b * P:(b + 1) * P], t_movp[:, o:o + n],
                        start=True, stop=True)
                    o += n

            t_env = envpool.tile([P, BLOCKS * F], bf16, tag="env")
            act_chain.append(nc.scalar.activation(
                t_env[:, :BLOCKS * w], p_q[:, :BLOCKS * w],
                mybir.ActivationFunctionType.Exp))
            t_k = kpool.tile([P, BLOCKS * F], i32, tag="k")
            t_frac = fracpool.tile([P, BLOCKS * F], f32, tag="frac")
            # DVE drains PSUM and converts at 2x from SBUF; the otherwise-idle
            # GPSIMD does the round-subtract in place (engine load balancing)
            nc.vector.tensor_copy(t_frac[:, :BLOCKS * w], p_p[:, :BLOCKS * w])
            nc.vector.tensor_copy(t_k[:, :BLOCKS * w], t_frac[:, :BLOCKS * w])
            nc.gpsimd.tensor_tensor(t_frac[:, :BLOCKS * w],
                                    t_frac[:, :BLOCKS * w],
                                    t_k[:, :BLOCKS * w],
                                    mybir.AluOpType.subtract)
            return t_env, t_frac

        def stage_b(i, t_env, t_frac):
            """Sin -> mult -> reduce-accumulate into p_out columns."""
            w = tf(i)
            t_sin = sinpool.tile([P, BLOCKS * F], bf16, tag="sin")
            act_chain.append(nc.scalar.activation(
                t_sin[:, :BLOCKS * w], t_frac[:, :BLOCKS * w],
                mybir.ActivationFunctionType.Sin, scale=2.0 * np.pi))
            t_m = mpool.tile([P, BLOCKS * F], bf16, tag="m")
            nc.vector.tensor_tensor(t_m[:, :BLOCKS * w], t_env[:, :BLOCKS * w],
                                    t_sin[:, :BLOCKS * w], mybir.AluOpType.mult)
            ncols = w // CHUNK
            # keep the accumulating pair adjacent: a start=True matmul resets
            # accumulation state bank-wide, so groups must not interleave
            for j in range(ncols):
                for b in range(BLOCKS):
                    c = (i * F) // CHUNK + j
                    nc.tensor.matmul(
                        p_out[:, c:c + 1],
                        t_m[:, b * w + j * CHUNK: b * w + (j + 1) * CHUNK],
                        t_ones[:],
                        start=(b == 0), stop=(b == BLOCKS - 1))

        prev = []
        i = 0
        while i < N_TILES:
            hi = min(i + BATCH, N_TILES)
            cur = [(j, *stage_a(j)) for j in range(i, hi)]
            for j, t_env, t_frac in prev:
                stage_b(j, t_env, t_frac)
            prev = cur
            i = hi
        for j, t_env, t_frac in prev:
            stage_b(j, t_env, t_frac)

        t_w = opool.tile([P, N_COLS], f32)
        act_chain.append(nc.scalar.copy(t_w[:], p_out[:, :N_COLS]))
        nc.sync.dma_start(d_out[:], t_w[:])
        for prev, nxt in zip(act_chain[:-1], act_chain[1:]):
            add_dep_helper(nxt.ins, prev.ins, sync=False,
                           reason="ACT table-set batching order")

    nc.compile()
    return nc


def _prepare_inputs(amplitude_logit, tau, omega_logit, sigma_logit,
                    phi_vector, gamma):
    """fp64 host prep -> per-core input maps."""
    al = amplitude_logit.astype(np.float64)
    tau = tau.astype(np.float64)
    ol = omega_logit.astype(np.float64)
    sl = sigma_logit.astype(np.float64)
    pv = phi_vector.astype(np.float64)
    gamma = gamma.astype(np.float64)

    ln_amp = np.where(al > 30, al, np.log(np.log1p(np.exp(al))))
    omega = (1.0 / (1.0 + np.exp(-ol))) * 0.99 * NYQUIST
    sigma = np.where(sl > 30, sl, np.log1p(np.exp(sl))) + SIGMA_OFFSET
    phi = np.arctan2(pv[:, 1], pv[:, 0])

    # shared moving rows (local time u = j/FS, exact grid)
    j = np.arange(F, dtype=np.float64)
    u = j / FS
    w2 = u * u
    u1, u2, u3 = _bsplit(u, 3)
    w1, w2b = _bsplit(w2, 2)
    one = np.ones(F, dtype=bft)
    mov_q = np.stack([one, one, u1, u1, u2, u2, w1, w1, w2b])
    mov_p = np.stack([one, one, u1, u1, u1, u2, u2, u3])

    t0s = (np.arange(N_TILES, dtype=np.float64) * F) / FS          # [I]
    in_maps = []
    for c in range(N_CORES):
        sel = slice(c * (N_ATOMS // N_CORES), (c + 1) * (N_ATOMS // N_CORES))
        tau_c, sig_c, om_c = tau[sel], sigma[sel], omega[sel]
        ga_c, phi_c, la_c = gamma[sel], phi[sel], ln_amp[sel]

        D = t0s[:, None] - tau_c[None, :]                           # [I, 256]
        inv_s2 = 1.0 / (sig_c * sig_c)
        c0 = -0.5 * D * D * inv_s2[None, :] + la_c[None, :]
        c1 = -D * inv_s2[None, :]
        c2 = np.broadcast_to(-0.5 * inv_s2[None, :], D.shape)
        Bc = om_c[None, :] + ga_c[None, :] * D / np.pi
        C = (om_c[None, :] * D + ga_c[None, :] * D * D / (2 * np.pi)
             + phi_c[None, :] / (2 * np.pi) + 0.25)
        C = C - np.round(C)

        c0_1, c0_2 = _bsplit(c0, 2)
        c1_1, c1_2 = _bsplit(c1, 2)
        c2_1, c2_2 = _bsplit(c2, 2)
        C1, C2 = _bsplit(C, 2)
        B1, B2, B3 = _bsplit(Bc, 3)

        stat_q = np.stack([c0_1, c0_2, c1_1, c1_2, c1_1, c1_2,
                           c2_1, c2_2, c2_1], axis=1)               # [I, 9, 256]
        stat_p = np.stack([C1, C2, B1, B2, B3, B1, B2, B1], axis=1)  # [I, 8, 256]
        in_maps.append({
            "mov_q": np.ascontiguousarray(mov_q),
            "mov_p": np.ascontiguousarray(mov_p),
            "stat_q": np.ascontiguousarray(stat_q),
            "stat_p": np.ascontiguousarray(stat_p),
        })
    return in_maps


def kernel(amplitude_logit, tau, omega_logit, sigma_logit, phi_vector, gamma, t):
    if "nc" not in _cache:
        _cache["nc"] = _build_program()
    nc = _cache["nc"]
    in_maps = _prepare_inputs(amplitude_logit, tau, omega_logit, sigma_logit,
                              phi_vector, gamma)
    res = run_bass_kernel_spmd(nc, in_maps, list(range(N_CORES)))
    total = np.zeros(T, dtype=np.float64)
    for r in res.results:
        wv = r["wave"].astype(np.float64)          # [P, N_COLS]
        total += wv.T.ravel()                      # sample s = c*128 + p
    return total.astype(np.float32)



# revision 3
# speedup vs baseline: 1.2730x; 1.2730x over previous
"""Gabor-atom additive audio synthesis on 8 Trainium2 NeuronCores.

Math: waveform[t] = sum_n amp_n * exp(-0.5*((t-tau_n)/sigma_n)^2)
                    * cos(2*pi*omega_n*(t-tau_n) + gamma_n*(t-tau_n)^2 + phi_n)
with N=2048 atoms, T=48000 samples (2s @ 24kHz).

Sharding: atoms sorted by per-sample phase rate beta=omega_eff/fs across the
whole problem, dealt to 8 cores in runs of 256 (2 blocks of 128 partitions).
The sum over atoms is permutation invariant, and sorting makes each block's
rate set compact.

Phase path (the key structure): within a 768-sample tile, the phase in
radians is  y = 2*pi*(saw_m(j) + C + r*j)  where m = round(768*beta) and
saw_m(j) = cfrac(m*j/768) is a centered sawtooth with period dividing 768,
so the same 768-wide sawtooth rows serve every tile. A PE matmul selects
each atom's sawtooth row one-hot and adds per-tile C (2 bf16 limbs) and
residual-rate rows r*j (|r| <= 1/1536, 3 limb-product rows). With C
re-centered per (atom, tile) on host, |y| <= ~7.9 rad < 3*pi, so a single
DVE ADD_RANGE_WRAP (in place in PSUM) lands the phase in [-pi, pi] and one
ACT Sin (the +pi/2 shift is folded into C so Sin yields the cosine)
produces the carrier. No activation-table swaps: ACT runs Sin only.

Envelope: amp*exp(-0.5*(dt/sigma)^2) is piecewise-linear over 384-sample
segments; the host sends knot values/slopes and the DVE expands them with
one tensor_scalar (iota*de + e0) per segment at 4x rate.

m = env*cos runs on DVE/GPSIMD (split for engine balance); a PE matmul
against ones reduces over the 128 atom partitions into output columns.
Host: fp64 coefficient prep, final 8-way partial sum.
"""
import numpy as np
import ml_dtypes
from contextlib import ExitStack

import concourse.bacc as bacc
import concourse.tile as tile
from concourse import mybir
from concourse.bass_utils import run_bass_kernel_spmd

# ---- problem constants (hardcoded; kernel.py must be self-contained) ----
FS = 24000.0
T = 48000
N_ATOMS = 2048
N_CORES = 8
NYQUIST = FS / 2.0
SIGMA_OFFSET = 1e-3

P = 128                      # partitions / atoms per block
BLOCKS = 2                   # atom blocks per core (256 atoms/core)
F = 768                      # time-tile width; sawtooth periods divide F
N_TILES = T // F + (1 if T % F else 0)      # 63 (62 full + 384 remainder)
REM = T - (N_TILES - 1) * F                 # 384
SEG = 384                    # envelope linear-interp segment
CHUNK = 128                  # reduce chunk (output column) width
N_COLS = T // CHUNK          # 375 output columns
KSAW = 75                    # sawtooth one-hot rows per block (padded)
K = KSAW + 5                 # + C1, C2 (ones rows) and r1*j1, r1*j2, r2*j1
NSEG = 2 * N_TILES           # envelope segments per block (126)
DMA_GRP = 8                  # stat DMA split: tiles per chunk

f32 = mybir.dt.float32
f16 = mybir.dt.float16
bf16 = mybir.dt.bfloat16
bft = ml_dtypes.bfloat16
TWO_PI = 2.0 * np.pi

_cache = {}


def _build_program():
    nc = bacc.Bacc("TRN2", target_bir_lowering=False, debug=False)

    d_saw = [nc.dram_tensor(f"saw{b}", [K, F], bf16, kind="ExternalInput").ap()
             for b in range(BLOCKS)]
    # stationary, k-major: [K, tile, block, atom]
    d_stat = nc.dram_tensor("stat", [K, N_TILES, BLOCKS, P], bf16,
                            kind="ExternalInput").ap()
    # envelope knots per block: e0 | de, each [P, NSEG] f32
    d_e0 = [nc.dram_tensor(f"e0_{b}", [P, NSEG], f32, kind="ExternalInput").ap()
            for b in range(BLOCKS)]
    d_de = [nc.dram_tensor(f"de_{b}", [P, NSEG], f32, kind="ExternalInput").ap()
            for b in range(BLOCKS)]
    d_iota = nc.dram_tensor("iota", [P, SEG], f16, kind="ExternalInput").ap()
    d_out = nc.dram_tensor("wave", [P, N_COLS], f32, kind="ExternalOutput").ap()

    with tile.TileContext(nc) as tc, ExitStack() as ctx:
        consts = ctx.enter_context(tc.tile_pool(name="consts", bufs=1))
        cospool = ctx.enter_context(tc.tile_pool(name="cos", bufs=4))
        envpool = ctx.enter_context(tc.tile_pool(name="env", bufs=4))
        mpool = ctx.enter_context(tc.tile_pool(name="m", bufs=4))
        opool = ctx.enter_context(tc.tile_pool(name="ocopy", bufs=1))
        ypool = ctx.enter_context(tc.tile_pool(name="yp", bufs=2, space="PSUM"))
        outpool = ctx.enter_context(tc.tile_pool(name="outp", bufs=1, space="PSUM"))

        # ---- resident constants ----
        t_iota = consts.tile([P, SEG], f16)
        nc.sync.dma_start(t_iota[:], d_iota[:])
        t_saw = []
        for b in range(BLOCKS):
            t = consts.tile([K, F], bf16, tag=f"saw{b}")
            nc.sync.dma_start(t[:], d_saw[b][:])
            t_saw.append(t)
        t_e0, t_de = [], []
        for b in range(BLOCKS):
            te = consts.tile([P, NSEG], f32, tag=f"e0_{b}")
            nc.gpsimd.dma_start(te[:], d_e0[b][:])
            t_e0.append(te)
            td = consts.tile([P, NSEG], f32, tag=f"de_{b}")
            nc.gpsimd.dma_start(td[:], d_de[b][:])
            t_de.append(td)
        t_ones = consts.tile([P, 1], bf16)
        nc.vector.memset(t_ones[:], 1.0)

        # all per-tile stationaries, resident; DMA in DMA_GRP-tile chunks
        t_stat = consts.tile([K, N_TILES * BLOCKS * P], bf16)
        i = 0
        while i < N_TILES:
            hi = min(i + DMA_GRP, N_TILES)
            nc.sync.dma_start(
                t_stat[:, i * BLOCKS * P: hi * BLOCKS * P],
                d_stat[:, i:hi])
            i = hi

        p_out = outpool.tile([P, 512], f32)

        def tf(i):
            return REM if i == N_TILES - 1 else F

        mult_on_pool = 0.0

        for i in range(N_TILES):
            w = tf(i)
            p_y = ypool.tile([P, BLOCKS * F], f32, tag="y")
            for b in range(BLOCKS):
                st = t_stat[:, (i * BLOCKS + b) * P: (i * BLOCKS + b + 1) * P]
                o = 0
                while o < w:
                    col = b * F + o
                    n = min(w - o, 512 - (col % 512))
                    nc.tensor.matmul(p_y[:, col: col + n],
                                     st, t_saw[b][:, o:o + n],
                                     start=True, stop=True)
                    o += n
            # phase wrap into [-pi, pi], in place in PSUM (one period is
            # enough: host centering guarantees |y| <= ~7.9 < 3*pi)
            t_cos = cospool.tile([P, BLOCKS * F], bf16, tag="cos")
            t_env = envpool.tile([P, BLOCKS * F], bf16, tag="env")
            t_m = mpool.tile([P, BLOCKS * F], bf16, tag="m")
            for b in range(BLOCKS):
                sl = slice(b * F, b * F + w)
                nc.vector.add_range_wrap(p_y[:, sl], p_y[:, sl],
                                         0.0, np.pi, TWO_PI)
                nc.scalar.activation(t_cos[:, sl], p_y[:, sl],
                                     mybir.ActivationFunctionType.Sin)
                for s in range(w // SEG):
                    seg = slice(b * F + s * SEG, b * F + (s + 1) * SEG)
                    nc.vector.tensor_scalar(
                        t_env[:, seg], t_iota[:],
                        t_de[b][:, 2 * i + s: 2 * i + s + 1],
                        t_e0[b][:, 2 * i + s: 2 * i + s + 1],
                        mybir.AluOpType.mult, mybir.AluOpType.add)
                # env*cos: split between DVE and GPSIMD for engine balance
                mult_on_pool += 0.73
                if mult_on_pool >= 1.0:
                    mult_on_pool -= 1.0
                    eng = nc.gpsimd
                else:
                    eng = nc.vector
                eng.tensor_tensor(t_m[:, sl], t_env[:, sl], t_cos[:, sl],
                                  mybir.AluOpType.mult)
            for j in range(w // CHUNK):
                c = (i * F) // CHUNK + j
                for b in range(BLOCKS):
                    nc.tensor.matmul(
                        p_out[:, c:c + 1],
                        t_m[:, b * F + j * CHUNK: b * F + (j + 1) * CHUNK],
                        t_ones[:],
                        start=(b == 0), stop=(b == BLOCKS - 1))

        t_w = opool.tile([P, N_COLS], f32)
        nc.scalar.copy(t_w[:], p_out[:, :N_COLS])
        nc.sync.dma_start(d_out[:], t_w[:])

    nc.compile()
    return nc


def _cfrac(x):
    return x - np.round(x)


def _prepare_inputs(amplitude_logit, tau, omega_logit, sigma_logit,
                    phi_vector, gamma):
    """fp64 host prep -> per-core input maps."""
    al = amplitude_logit.astype(np.float64)
    tau = tau.astype(np.float64)
    ol = omega_logit.astype(np.float64)
    sl = sigma_logit.astype(np.float64)
    pv = phi_vector.astype(np.float64)
    gamma = gamma.astype(np.float64)

    amp = np.where(al > 30, al, np.log1p(np.exp(al)))
    omega = (1.0 / (1.0 + np.exp(-ol))) * 0.99 * NYQUIST
    sigma = np.where(sl > 30, sl, np.log1p(np.exp(sl))) + SIGMA_OFFSET
    phi = np.arctan2(pv[:, 1], pv[:, 0])

    # sort atoms by center phase rate (cycles/sample); deal runs of 256/core
    beta_mid = (omega + gamma * (1.0 - tau) / np.pi) / FS
    order = np.argsort(beta_mid)
    amp, tau_s, omega_s = amp[order], tau[order], omega[order]
    sigma_s, phi_s, gamma_s = sigma[order], phi[order], gamma[order]
    m_all = np.round(F * beta_mid[order]).astype(np.int64)

    jl = np.arange(F, dtype=np.float64)
    iota = np.broadcast_to(np.arange(SEG, dtype=np.float64), (P, SEG))
    t0s = np.arange(N_TILES, dtype=np.float64) * F / FS        # [I]
    # envelope knot times: tile starts + mid + final end
    kn = np.arange(2 * N_TILES + 1, dtype=np.float64) * SEG / FS

    in_maps = []
    for c in range(N_CORES):
        saws, e0s, des = [], [], []
        stat = np.zeros((K, N_TILES, BLOCKS, P), dtype=bft)
        for b in range(BLOCKS):
            sel = slice(c * BLOCKS * P + b * P, c * BLOCKS * P + (b + 1) * P)
            am, ta, om = amp[sel], tau_s[sel], omega_s[sel]
            sg, ph, ga = sigma_s[sel], phi_s[sel], gamma_s[sel]
            m = m_all[sel]

            ms = np.unique(m)
            ms = ms[ms != 0]
            assert len(ms) <= KSAW, f"block saw rows {len(ms)} > {KSAW}"
            saw = np.zeros((K, F), dtype=bft)
            saw[:len(ms)] = (TWO_PI * _cfrac(ms[:, None] * jl[None, :] / F)
                             ).astype(bft)
            # residual-rate moving rows: ones, ones, j1, j2, j1
            j1 = jl.astype(bft)
            j2 = (jl - j1.astype(np.float64)).astype(bft)
            saw[KSAW + 0] = bft(1.0)
            saw[KSAW + 1] = bft(1.0)
            saw[KSAW + 2] = j1
            saw[KSAW + 3] = j2
            saw[KSAW + 4] = j1
            saws.append(np.ascontiguousarray(saw))

            row_of = {mm: r for r, mm in enumerate(ms)}
            rows = np.array([row_of.get(mm, -1) for mm in m])  # [P]

            # per (tile, atom): phase at tile start, rate, residual
            D = t0s[:, None] - ta[None, :]                      # [I, P]
            Y0 = (om[None, :] * D + ga[None, :] * D * D / TWO_PI
                  + ph[None, :] / TWO_PI + 0.25)               # cycles
            beta_t = (om[None, :] + ga[None, :] * D / np.pi) / FS
            r = beta_t - m[None, :] / F                        # [I, P]
            assert np.abs(r).max() < 1.0 / 1536 + 1e-5

            C_raw = _cfrac(Y0)
            mid = C_raw + r * (F - 1) / 2.0
            Cc = C_raw - np.round(mid)                         # center |y|
            C_rad = TWO_PI * Cc
            C1 = C_rad.astype(bft)
            C2 = (C_rad - C1.astype(np.float64)).astype(bft)
            r_rad = TWO_PI * r
            r1 = r_rad.astype(bft)
            r2 = (r_rad - r1.astype(np.float64)).astype(bft)

            onehot = np.zeros((KSAW, N_TILES, P), dtype=bft)
            for p in range(P):
                if rows[p] >= 0:
                    onehot[rows[p], :, p] = bft(1.0)
            stat[:KSAW, :, b, :] = onehot
            stat[KSAW + 0, :, b, :] = C1
            stat[KSAW + 1, :, b, :] = C2
            stat[KSAW + 2, :, b, :] = r1
            stat[KSAW + 3, :, b, :] = r1
            stat[KSAW + 4, :, b, :] = r2

            # envelope knots: e(t) at segment boundaries -> e0, de per seg
            dk = kn[:, None] - ta[None, :]                      # [2I+1, P]
            ev = am[None, :] * np.exp(-0.5 * (dk / sg[None, :]) ** 2)
            e0 = ev[:-1]                                        # [2I, P]
            de = (ev[1:] - ev[:-1]) / SEG
            e0s.append(np.ascontiguousarray(e0.T.astype(np.float32)))
            des.append(np.ascontiguousarray(de.T.astype(np.float32)))

        im = {
            "stat": np.ascontiguousarray(stat),
            "iota": np.ascontiguousarray(iota.astype(ml_dtypes.float16
                                                     if hasattr(ml_dtypes, 'float16')
                                                     else np.float16)),
        }
        for b in range(BLOCKS):
            im[f"saw{b}"] = saws[b]
            im[f"e0_{b}"] = e0s[b]
            im[f"de_{b}"] = des[b]
        in_maps.append(im)
    return in_maps


def kernel(amplitude_logit, tau, omega_logit, sigma_logit, phi_vector, gamma, t):
    if "nc" not in _cache:
        _cache["nc"] = _build_program()
    nc = _cache["nc"]
    in_maps = _prepare_inputs(amplitude_logit, tau, omega_logit, sigma_logit,
                              phi_vector, gamma)
    res = run_bass_kernel_spmd(nc, in_maps, list(range(N_CORES)))
    total = np.zeros(T, dtype=np.float64)
    for r in res.results:
        wv = r["wave"].astype(np.float64)          # [P, N_COLS]
        total += wv.T.ravel()                      # sample s = c*128 + p
    return total.astype(np.float32)


# revision 4
# speedup vs baseline: 1.3854x; 1.0884x over previous
"""Gabor-atom additive audio synthesis on 8 Trainium2 NeuronCores.

Math: waveform[t] = sum_n amp_n * exp(-0.5*((t-tau_n)/sigma_n)^2)
                    * cos(2*pi*omega_n*(t-tau_n) + gamma_n*(t-tau_n)^2 + phi_n)
with N=2048 atoms, T=48000 samples (2s @ 24kHz).

Sharding: atoms sorted by per-sample phase rate beta=omega_eff/fs across the
whole problem, dealt to 8 cores in runs of 256 (2 blocks of 128 partitions).
The sum over atoms is permutation invariant, and sorting makes each block's
rate set compact.

Phase path (the key structure): within a 768-sample tile, the phase in
radians is  y = 2*pi*(saw_m(j) + C + r*j)  where m = round(768*beta) and
saw_m(j) = cfrac(m*j/768) is a centered sawtooth with period dividing 768,
so the same 768-wide sawtooth rows serve every tile. A PE matmul selects
each atom's sawtooth row one-hot and adds per-tile C (2 bf16 limbs) and
residual-rate rows r*j (|r| <= 1/1536, 3 limb-product rows). With C
re-centered per (atom, tile) on host, |y| <= ~7.9 rad < 3*pi, so a single
DVE ADD_RANGE_WRAP (in place in PSUM) lands the phase in [-pi, pi] and one
ACT Sin (the +pi/2 shift is folded into C so Sin yields the cosine)
produces the carrier. No activation-table swaps: ACT runs Sin only.

Envelope: amp*exp(-0.5*(dt/sigma)^2) is piecewise-linear over 384-sample
segments; the host sends knot values/slopes and the DVE expands them with
one tensor_scalar (iota*de + e0) per segment at 4x rate.

m = env*cos runs on DVE/GPSIMD (split for engine balance); a PE matmul
against ones reduces over the 128 atom partitions into output columns.
Host: fp64 coefficient prep, final 8-way partial sum.
"""
import numpy as np
import ml_dtypes
from contextlib import ExitStack

import concourse.bacc as bacc
import concourse.tile as tile
from concourse import mybir
from concourse.bass_utils import run_bass_kernel_spmd

# ---- problem constants (hardcoded; kernel.py must be self-contained) ----
FS = 24000.0
T = 48000
N_ATOMS = 2048
N_CORES = 8
NYQUIST = FS / 2.0
SIGMA_OFFSET = 1e-3

P = 128                      # partitions / atoms per block
BLOCKS = 2                   # atom blocks per core (256 atoms/core)
F = 768                      # time-tile width; sawtooth periods divide F
N_TILES = T // F + (1 if T % F else 0)      # 63 (62 full + 384 remainder)
REM = T - (N_TILES - 1) * F                 # 384
SEG = 384                    # envelope linear-interp segment
CHUNK = 128                  # reduce chunk (output column) width
N_COLS = T // CHUNK          # 375 output columns
KSAW = 75                    # sawtooth one-hot rows per block (padded)
K = KSAW + 5                 # + C1, C2 (ones rows) and r1*j1, r1*j2, r2*j1
NSEG = 2 * N_TILES           # envelope segments per block (126)
DMA_GRP = 8                  # stat DMA split: tiles per chunk

f32 = mybir.dt.float32
f16 = mybir.dt.float16
bf16 = mybir.dt.bfloat16
bft = ml_dtypes.bfloat16
TWO_PI = 2.0 * np.pi

_cache = {}


def _build_program():
    nc = bacc.Bacc("TRN2", target_bir_lowering=False, debug=False)

    d_saw = [nc.dram_tensor(f"saw{b}", [K, F], bf16, kind="ExternalInput").ap()
             for b in range(BLOCKS)]
    # stationary, k-major: [K, tile, block, atom]
    d_stat = nc.dram_tensor("stat", [K, N_TILES, BLOCKS, P], bf16,
                            kind="ExternalInput").ap()
    # envelope knots per block: e0 | de, each [P, NSEG] f32
    d_e0 = [nc.dram_tensor(f"e0_{b}", [P, NSEG], f32, kind="ExternalInput").ap()
            for b in range(BLOCKS)]
    d_de = [nc.dram_tensor(f"de_{b}", [P, NSEG], f32, kind="ExternalInput").ap()
            for b in range(BLOCKS)]
    d_iota = nc.dram_tensor("iota", [P, SEG], f16, kind="ExternalInput").ap()
    d_out = nc.dram_tensor("wave", [P, N_COLS], f32, kind="ExternalOutput").ap()

    with tile.TileContext(nc) as tc, ExitStack() as ctx:
        consts = ctx.enter_context(tc.tile_pool(name="consts", bufs=1))
        cospool = ctx.enter_context(tc.tile_pool(name="cos", bufs=4))
        envpool = ctx.enter_context(tc.tile_pool(name="env", bufs=4))
        mpool = ctx.enter_context(tc.tile_pool(name="m", bufs=4))
        opool = ctx.enter_context(tc.tile_pool(name="ocopy", bufs=1))
        ypool = ctx.enter_context(tc.tile_pool(name="yp", bufs=2, space="PSUM"))
        outpool = ctx.enter_context(tc.tile_pool(name="outp", bufs=1, space="PSUM"))

        # ---- resident constants ----
        t_iota = consts.tile([P, SEG], f16)
        nc.sync.dma_start(t_iota[:], d_iota[:])
        t_saw = []
        for b in range(BLOCKS):
            t = consts.tile([K, F], bf16, tag=f"saw{b}")
            nc.sync.dma_start(t[:], d_saw[b][:])
            t_saw.append(t)
        t_e0, t_de = [], []
        for b in range(BLOCKS):
            te = consts.tile([P, NSEG], f32, tag=f"e0_{b}")
            nc.gpsimd.dma_start(te[:], d_e0[b][:])
            t_e0.append(te)
            td = consts.tile([P, NSEG], f32, tag=f"de_{b}")
            nc.gpsimd.dma_start(td[:], d_de[b][:])
            t_de.append(td)
        t_ones = consts.tile([P, 1], bf16)
        nc.vector.memset(t_ones[:], 1.0)

        # all per-tile stationaries, resident; DMA in DMA_GRP-tile chunks
        t_stat = consts.tile([K, N_TILES * BLOCKS * P], bf16)
        i = 0
        while i < N_TILES:
            hi = min(i + DMA_GRP, N_TILES)
            nc.sync.dma_start(
                t_stat[:, i * BLOCKS * P: hi * BLOCKS * P],
                d_stat[:, i:hi])
            i = hi

        p_out = outpool.tile([P, 512], f32)

        def tf(i):
            return REM if i == N_TILES - 1 else F

        mult_on_pool = 0.0

        for i in range(N_TILES):
            w = tf(i)
            p_y = ypool.tile([P, BLOCKS * F], f32, tag="y")
            for b in range(BLOCKS):
                st = t_stat[:, (i * BLOCKS + b) * P: (i * BLOCKS + b + 1) * P]
                o = 0
                while o < w:
                    col = b * F + o
                    n = min(w - o, 512 - (col % 512))
                    nc.tensor.matmul(p_y[:, col: col + n],
                                     st, t_saw[b][:, o:o + n],
                                     start=True, stop=True)
                    o += n
            # phase wrap into [-pi, pi], in place in PSUM (one period is
            # enough: host centering guarantees |y| <= ~7.9 < 3*pi)
            t_cos = cospool.tile([P, BLOCKS * F], bf16, tag="cos")
            t_env = envpool.tile([P, BLOCKS * F], bf16, tag="env")
            t_m = mpool.tile([P, BLOCKS * F], bf16, tag="m")
            spans = ([slice(0, BLOCKS * F)] if w == F else
                     [slice(b * F, b * F + w) for b in range(BLOCKS)])
            for sl in spans:
                nc.vector.add_range_wrap(p_y[:, sl], p_y[:, sl],
                                         0.0, np.pi, TWO_PI)
                nc.scalar.activation(t_cos[:, sl], p_y[:, sl],
                                     mybir.ActivationFunctionType.Sin)
            # envelope expansion: per-segment affine of iota; split between
            # DVE tensor_scalar (4x) and ACT Identity (same trig table set)
            # for engine balance
            for b in range(BLOCKS):
                for s in range(w // SEG):
                    seg = slice(b * F + s * SEG, b * F + (s + 1) * SEG)
                    de_ap = t_de[b][:, 2 * i + s: 2 * i + s + 1]
                    e0_ap = t_e0[b][:, 2 * i + s: 2 * i + s + 1]
                    if (i * 2 + b * 2 + s) % 8 in (3, 7, 5):
                        nc.scalar.activation(
                            t_env[:, seg], t_iota[:],
                            mybir.ActivationFunctionType.Identity,
                            bias=e0_ap, scale=de_ap)
                    else:
                        nc.vector.tensor_scalar(
                            t_env[:, seg], t_iota[:], de_ap, e0_ap,
                            mybir.AluOpType.mult, mybir.AluOpType.add)
            # env*cos: split between DVE and GPSIMD for engine balance
            mult_on_pool += 0.75
            if mult_on_pool >= 1.0:
                mult_on_pool -= 1.0
                eng = nc.gpsimd
            else:
                eng = nc.vector
            for sl in spans:
                eng.tensor_tensor(t_m[:, sl], t_env[:, sl], t_cos[:, sl],
                                  mybir.AluOpType.mult)
            for j in range(w // CHUNK):
                c = (i * F) // CHUNK + j
                for b in range(BLOCKS):
                    nc.tensor.matmul(
                        p_out[:, c:c + 1],
                        t_m[:, b * F + j * CHUNK: b * F + (j + 1) * CHUNK],
                        t_ones[:],
                        start=(b == 0), stop=(b == BLOCKS - 1))

        t_w = opool.tile([P, N_COLS], f32)
        nc.scalar.copy(t_w[:], p_out[:, :N_COLS])
        nc.sync.dma_start(d_out[:], t_w[:])

    nc.compile()
    return nc


def _cfrac(x):
    return x - np.round(x)


def _prepare_inputs(amplitude_logit, tau, omega_logit, sigma_logit,
                    phi_vector, gamma):
    """fp64 host prep -> per-core input maps."""
    al = amplitude_logit.astype(np.float64)
    tau = tau.astype(np.float64)
    ol = omega_logit.astype(np.float64)
    sl = sigma_logit.astype(np.float64)
    pv = phi_vector.astype(np.float64)
    gamma = gamma.astype(np.float64)

    amp = np.where(al > 30, al, np.log1p(np.exp(al)))
    omega = (1.0 / (1.0 + np.exp(-ol))) * 0.99 * NYQUIST
    sigma = np.where(sl > 30, sl, np.log1p(np.exp(sl))) + SIGMA_OFFSET
    phi = np.arctan2(pv[:, 1], pv[:, 0])

    # sort atoms by center phase rate (cycles/sample); deal runs of 256/core
    beta_mid = (omega + gamma * (1.0 - tau) / np.pi) / FS
    order = np.argsort(beta_mid)
    amp, tau_s, omega_s = amp[order], tau[order], omega[order]
    sigma_s, phi_s, gamma_s = sigma[order], phi[order], gamma[order]
    m_all = np.round(F * beta_mid[order]).astype(np.int64)

    jl = np.arange(F, dtype=np.float64)
    iota = np.broadcast_to(np.arange(SEG, dtype=np.float64), (P, SEG))
    t0s = np.arange(N_TILES, dtype=np.float64) * F / FS        # [I]
    # envelope knot times: tile starts + mid + final end
    kn = np.arange(2 * N_TILES + 1, dtype=np.float64) * SEG / FS

    in_maps = []
    for c in range(N_CORES):
        saws, e0s, des = [], [], []
        stat = np.zeros((K, N_TILES, BLOCKS, P), dtype=bft)
        for b in range(BLOCKS):
            sel = slice(c * BLOCKS * P + b * P, c * BLOCKS * P + (b + 1) * P)
            am, ta, om = amp[sel], tau_s[sel], omega_s[sel]
            sg, ph, ga = sigma_s[sel], phi_s[sel], gamma_s[sel]
            m = m_all[sel]

            ms = np.unique(m)
            ms = ms[ms != 0]
            assert len(ms) <= KSAW, f"block saw rows {len(ms)} > {KSAW}"
            saw = np.zeros((K, F), dtype=bft)
            saw[:len(ms)] = (TWO_PI * _cfrac(ms[:, None] * jl[None, :] / F)
                             ).astype(bft)
            # residual-rate moving rows: ones, ones, j1, j2, j1
            j1 = jl.astype(bft)
            j2 = (jl - j1.astype(np.float64)).astype(bft)
            saw[KSAW + 0] = bft(1.0)
            saw[KSAW + 1] = bft(1.0)
            saw[KSAW + 2] = j1
            saw[KSAW + 3] = j2
            saw[KSAW + 4] = j1
            saws.append(np.ascontiguousarray(saw))

            row_of = {mm: r for r, mm in enumerate(ms)}
            rows = np.array([row_of.get(mm, -1) for mm in m])  # [P]

            # per (tile, atom): phase at tile start, rate, residual
            D = t0s[:, None] - ta[None, :]                      # [I, P]
            Y0 = (om[None, :] * D + ga[None, :] * D * D / TWO_PI
                  + ph[None, :] / TWO_PI + 0.25)               # cycles
            beta_t = (om[None, :] + ga[None, :] * D / np.pi) / FS
            r = beta_t - m[None, :] / F                        # [I, P]
            assert np.abs(r).max() < 1.0 / 1536 + 1e-5

            C_raw = _cfrac(Y0)
            mid = C_raw + r * (F - 1) / 2.0
            Cc = C_raw - np.round(mid)                         # center |y|
            C_rad = TWO_PI * Cc
            C1 = C_rad.astype(bft)
            C2 = (C_rad - C1.astype(np.float64)).astype(bft)
            r_rad = TWO_PI * r
            r1 = r_rad.astype(bft)
            r2 = (r_rad - r1.astype(np.float64)).astype(bft)

            onehot = np.zeros((KSAW, N_TILES, P), dtype=bft)
            for p in range(P):
                if rows[p] >= 0:
                    onehot[rows[p], :, p] = bft(1.0)
            stat[:KSAW, :, b, :] = onehot
            stat[KSAW + 0, :, b, :] = C1
            stat[KSAW + 1, :, b, :] = C2
            stat[KSAW + 2, :, b, :] = r1
            stat[KSAW + 3, :, b, :] = r1
            stat[KSAW + 4, :, b, :] = r2

            # envelope knots: e(t) at segment boundaries -> e0, de per seg
            dk = kn[:, None] - ta[None, :]                      # [2I+1, P]
            ev = am[None, :] * np.exp(-0.5 * (dk / sg[None, :]) ** 2)
            e0 = ev[:-1]                                        # [2I, P]
            de = (ev[1:] - ev[:-1]) / SEG
            e0s.append(np.ascontiguousarray(e0.T.astype(np.float32)))
            des.append(np.ascontiguousarray(de.T.astype(np.float32)))

        im = {
            "stat": np.ascontiguousarray(stat),
            "iota": np.ascontiguousarray(iota.astype(ml_dtypes.float16
                                                     if hasattr(ml_dtypes, 'float16')
                                                     else np.float16)),
        }
        for b in range(BLOCKS):
            im[f"saw{b}"] = saws[b]
            im[f"e0_{b}"] = e0s[b]
            im[f"de_{b}"] = des[b]
        in_maps.append(im)
    return in_maps


def kernel(amplitude_logit, tau, omega_logit, sigma_logit, phi_vector, gamma, t):
    if "nc" not in _cache:
        _cache["nc"] = _build_program()
    nc = _cache["nc"]
    in_maps = _prepare_inputs(amplitude_logit, tau, omega_logit, sigma_logit,
                              phi_vector, gamma)
    res = run_bass_kernel_spmd(nc, in_maps, list(range(N_CORES)))
    total = np.zeros(T, dtype=np.float64)
    for r in res.results:
        wv = r["wave"].astype(np.float64)          # [P, N_COLS]
        total += wv.T.ravel()                      # sample s = c*128 + p
    return total.astype(np.float32)
